# revision 13
# baseline (speedup 1.0000x reference)
"""ChessBoardAttention Trainium2 kernel.

Full inputs -> full output. The 32 independent (batch, chessboard-offset)
attention problems are sharded 4-per-core across 8 NeuronCores; the
chessboard gather/scatter is pure data movement done host-side as part of
sharding.

Per-core device kernel, per problem (x_off: [64, 2304] f32):
  q/k = relu(Wqk @ x + b)            [8, L]   (bias via ones-row in x)
  vT  = relu(gamma * (x_chunk.T @ Wv.T + bv))  [L, 64] fp8e4, computed
        transposed in 128-row chunks; column 64 holds exact 1.0 so AV row 64
        accumulates Z = sum_m p (gamma is folded into Wv/bv host-side).
  S_T[m, l] = k[:,m-chunk].T @ q     fp32r scores, transposed so the AV
                                     contraction runs over PSUM partitions
  P_T = exp(S_T) -> fp8e4            split across engines: Act exp reads
        PSUM directly (13 of 18 m-chunks); the other 5 chunks go
        DVE copy PSUM->SBUF then GPSIMD pow(e, s) (gpsimd cannot read PSUM).
  AV: fp8e4 DoubleRow matmuls, 2 m-chunks per instruction (the pair is an
      AP dim: lhsT [128, 2, 65], rhs [128, 2, w]), PSUM fp32 accumulate.
  out = AV[0:64] * recip(Z) + x      (recip on DVE; Z broadcast along
                                      partitions via a DRAM roundtrip;
                                      normalize mult + residual on GPSIMD)
"""

import numpy as np

import concourse.bass as bass
import concourse.tile as tile
from concourse import mybir
from concourse.bass_utils import run_bass_kernel_spmd

F32 = mybir.dt.float32
F32R = mybir.dt.float32r
F8 = mybir.dt.float8e4
AT = mybir.AluOpType
DR = mybir.MatmulPerfMode.DoubleRow
E_CONST = float(np.e)

B, C, H, W = 2, 64, 192, 192
C8 = 8
HQ, WQ = H // 4, W // 4
L = HQ * WQ            # 2304
NPROB = 4              # problems per core
NCORES = 8
NM = L // 128          # 18 m-chunks of 128
LBLOCKS = [(0, 512), (512, 512), (1024, 512), (1536, 512), (2048, 256)]
VS = 80                # vT chunk stride (65 used; DoubleRow needs step%16==0)
SGRP = 3               # m-chunks per score-psum group (3 banks)
NGRP = NM // SGRP      # 6 groups
NACT = 13              # m-chunks exp'd on Act (rest: DVE copy + gpsimd pow)


def split_drain_waits(nc, keep=1):
    """This walrus build rejects instructions carrying more than a couple of
    sem-waits. Move excess waits onto single-wait DRAIN instructions inserted
    just before the offender on the same engine (drains with one wait are
    known-good through codegen)."""
    for f in nc.m.functions:
        for bb in f.blocks:
            insts = bb.instructions
            idx = 0
            while idx < len(insts):
                i = insts[idx]
                si = i.sync_info
                lim = keep
                if si is not None and si.on_wait and len(si.on_wait) > lim:
                    waits = list(si.on_wait)
                    si.on_wait = waits[-lim:]
                    for k, wt in enumerate(waits[:-lim]):
                        d = mybir.InstDrain(
                            name=f"{i.name}_wsplit{k}", ins=[], outs=[],
                            bass_is_fusable=False,
                        )
                        d.engine = i.engine
                        d.sync_info = mybir.SyncInfo(on_wait=[wt], on_update=[])
                        nc.register_instruction(d)
                        insts.insert(idx, d)
                        idx += 1
                idx += 1


def build_module():
    nc = bass.Bass("TRN2", target_bir_lowering=False, debug=False,
                   enable_asserts=False)
    xoffs = nc.dram_tensor("xoffs", [NPROB, C + 1, L], F32R, kind="ExternalInput").ap()
    xres = nc.dram_tensor("xres", [NPROB, C, L], F32, kind="ExternalInput").ap()
    wqk = nc.dram_tensor("wqk", [C + 1, 40], F32R, kind="ExternalInput").ap()
    wv = nc.dram_tensor("wv", [C + 1, C], F32R, kind="ExternalInput").ap()
    gam = nc.dram_tensor("gam", [128, 1], F32, kind="ExternalInput").ap()
    out_d = nc.dram_tensor("out", [NPROB, C, L], F32, kind="ExternalOutput").ap()

    with tile.TileContext(nc) as tc:
        with (
            tc.tile_pool(name="singles", bufs=1) as singles,
            tc.tile_pool(name="io", bufs=2) as io,
            tc.tile_pool(name="qk", bufs=2) as qkp,
            tc.tile_pool(name="vt", bufs=2) as vtp,
            tc.tile_pool(name="pt", bufs=2) as ptp,
            tc.tile_pool(name="scr", bufs=2) as scrp,
            tc.tile_pool(name="small", bufs=2) as smallp,
            tc.tile_pool(name="ps_s", bufs=2, space="PSUM") as ps_s_p,
            tc.tile_pool(name="ps_o", bufs=2, space="PSUM") as ps_o_p,
            tc.tile_pool(name="dram", bufs=2, space="DRAM") as dramp,
        ):
            wqk_sb = singles.tile([C + 1, 40], F32R)
            nc.sync.dma_start(out=wqk_sb, in_=wqk)
            wv_sb = singles.tile([C + 1, C], F32R)
            nc.sync.dma_start(out=wv_sb, in_=wv)
            e_sb = singles.tile([128, SGRP * 512], F32)
            nc.gpsimd.memset(e_sb, E_CONST)
            e3 = e_sb.rearrange("p (n w) -> p n w", w=512)
            gam_sb = singles.tile([128, 1], F32)
            nc.sync.dma_start(out=gam_sb, in_=gam)

            for p in range(NPROB):
                # ---- load x (ones row for the bias trick baked in host-side) ----
                x_sb = io.tile([C + 1, L], F32R, tag="x")
                nc.sync.dma_start(out=x_sb, in_=xoffs[p])
                xres_sb = io.tile([C, L], F32, tag="xres")
                nc.sync.dma_start(out=xres_sb, in_=xres[p])

                # ---- q/k projection: [16, L] = wqk.T @ x_aug, relu ----
                q_sb = qkp.tile([C8, L], F32R, tag="q")
                k_sb = qkp.tile([C8, L], F32R, tag="k")
                for st, w in LBLOCKS:
                    ps = ps_o_p.tile([128, 512], F32, tag="o")
                    nc.tensor.matmul(
                        ps[:40, :w], lhsT=wqk_sb, rhs=x_sb[:, st : st + w],
                        start=True, stop=True,
                    )
                    nc.vector.tensor_scalar_max(
                        out=q_sb[:, st : st + w], in0=ps[0:C8, :w], scalar1=0.0)
                    nc.vector.tensor_scalar_max(
                        out=k_sb[:, st : st + w], in0=ps[32:40, :w], scalar1=0.0)

                # ---- v projection, transposed: vT[m, c] fp8, 128-row chunks ----
                vT_sb = vtp.tile([128, NM * VS], F8, tag="vt")
                vT3 = vT_sb.rearrange("p (n c) -> p n c", c=VS)
                nc.gpsimd.memset(vT3[:, :, 64:65], 1.0)
                for g in range(3):
                    cnt = 6 if g < 2 else NM - 12
                    ps = ps_o_p.tile([128, 512], F32, tag="o")
                    for j in range(cnt):
                        mc = g * 6 + j
                        nc.tensor.matmul(
                            ps[:, j * C : (j + 1) * C],
                            lhsT=x_sb[:, mc * 128 : (mc + 1) * 128],
                            rhs=wv_sb, start=True, stop=True,
                        )
                    ps3 = ps.rearrange("p (n c) -> p n c", c=C)
                    nc.vector.tensor_scalar(
                        out=vT3[:, g * 6 : g * 6 + cnt, 0:C],
                        in0=ps3[:, 0:cnt, :], scalar1=0.0, scalar2=gam_sb[:, 0:1],
                        op0=AT.max, op1=AT.mult)

                # ---- attention over l-blocks ----
                av_sb = io.tile([C + 1, L], F32, tag="av")
                for st, w in LBLOCKS:
                    pT_sb = ptp.tile([128, NM * 512], F8, tag="pt")
                    pT3 = pT_sb.rearrange("p (n c) -> p n c", c=512)
                    scratch = scrp.tile([128, (NM - NACT) * 512], F32, tag="scr")
                    sc3 = scratch.rearrange("p (n c) -> p n c", c=512)
                    for g in range(NGRP):
                        ps_s = ps_s_p.tile([128, SGRP * 512], F32, tag="s")
                        for j in range(SGRP):
                            mc = g * SGRP + j
                            nc.tensor.matmul(
                                ps_s[:, j * 512 : j * 512 + w],
                                lhsT=k_sb[:, mc * 128 : (mc + 1) * 128],
                                rhs=q_sb[:, st : st + w],
                                start=True, stop=True,
                            )
                        ps_s3 = ps_s.rearrange("p (n c) -> p n c", c=512)
                        lo = g * SGRP
                        hi = lo + SGRP
                        na = min(max(NACT - lo, 0), SGRP)  # chunks for Act
                        if na > 0:
                            nc.scalar.activation(
                                out=pT3[:, lo : lo + na, :w],
                                in_=ps_s3[:, 0:na, :w],
                                func=mybir.ActivationFunctionType.Exp,
                            )
                        if na < SGRP:
                            # pool path: DVE copy PSUM->SBUF, gpsimd pow
                            sl = lo + na - NACT
                            nc.vector.tensor_copy(
                                sc3[:, sl : sl + SGRP - na, :w],
                                ps_s3[:, na:SGRP, :w])
                            nc.gpsimd.tensor_tensor(
                                out=pT3[:, lo + na : hi, :w],
                                in0=e3[:, 0 : SGRP - na, :w],
                                in1=sc3[:, sl : sl + SGRP - na, :w],
                                op=AT.pow)
                    ps_o = ps_o_p.tile([C + 1, 512], F32, tag="o")
                    for jp in range(NM // 2):
                        nc.tensor.matmul(
                            ps_o[:, :w],
                            lhsT=vT3[:, 2 * jp : 2 * jp + 2, 0 : C + 1],
                            rhs=pT3[:, 2 * jp : 2 * jp + 2, :w],
                            start=(jp == 0), stop=(jp == NM // 2 - 1),
                            perf_mode=DR,
                        )
                    nc.vector.tensor_copy(av_sb[:, st : st + w], ps_o[:, :w])

                # ---- normalize (row C of av_sb is Z), residual ----
                nc.vector.reciprocal(
                    out=av_sb[C : C + 1, :], in_=av_sb[C : C + 1, :])
                dram_rec = dramp.tile([1, L], F32, tag="drec")
                nc.sync.dma_start(out=dram_rec, in_=av_sb[C : C + 1, :])
                rec_rep = smallp.tile([C, L], F32, tag="recrep")
                rec_b = bass.AP(
                    tensor=dram_rec.tensor, offset=dram_rec.offset,
                    ap=[[0, C]] + list(dram_rec.ap)[1:])
                nc.sync.dma_start(out=rec_rep, in_=rec_b)
                nc.gpsimd.tensor_tensor(
                    out=av_sb[0:C, :], in0=av_sb[0:C, :], in1=rec_rep, op=AT.mult)
                nc.gpsimd.tensor_tensor(
                    out=av_sb[0:C, :], in0=av_sb[0:C, :], in1=xres_sb, op=AT.add)
                nc.sync.dma_start(out=out_d[p], in_=av_sb[0:C, :])

    split_drain_waits(nc)
    return nc


_NC = None


def _get_nc():
    global _NC
    if _NC is None:
        _NC = build_module()
    return _NC


def make_in_maps(x, Wq, bq, Wk, bk, Wv, bv, gamma):
    x = np.asarray(x, np.float32)
    xoff = (
        x.reshape(B, C, HQ, 4, WQ, 4)
        .transpose(0, 3, 5, 1, 2, 4)
        .reshape(B * 16, C, L)
    )
    ones = np.ones((B * 16, 1, L), np.float32)
    xoff_aug = np.concatenate([xoff, ones], 1)   # [32, C+1, L]
    wqk = np.zeros((C + 1, 40), np.float32)   # q -> psum parts 0-7, k -> 32-39
    wqk[:C, 0:C8] = np.asarray(Wq).T
    wqk[C, 0:C8] = np.asarray(bq)
    wqk[:C, 32:40] = np.asarray(Wk).T
    wqk[C, 32:40] = np.asarray(bk)
    g = np.float32(np.asarray(gamma).reshape(-1)[0])
    gam_col = np.full((128, 1), g, np.float32)
    wv = np.concatenate(
        [np.asarray(Wv).T, np.asarray(bv)[None, :]], 0
    ).astype(np.float32)                      # [65, 64]
    in_maps = []
    for c in range(NCORES):
        in_maps.append(
            {
                "xoffs": np.ascontiguousarray(xoff_aug[c * NPROB : (c + 1) * NPROB]),
                "xres": np.ascontiguousarray(xoff[c * NPROB : (c + 1) * NPROB]),
                "wqk": wqk,
                "wv": wv,
                "gam": gam_col,
            }
        )
    return in_maps


def unshard(results):
    outp = np.concatenate([results[c]["out"] for c in range(NCORES)], 0)
    return (
        outp.reshape(B, 4, 4, C, HQ, WQ)
        .transpose(0, 3, 4, 1, 5, 2)
        .reshape(B, C, H, W)
        .astype(np.float32)
    )


def kernel(**inputs):
    nc = _get_nc()
    in_maps = make_in_maps(**inputs)
    res = run_bass_kernel_spmd(nc, in_maps, list(range(NCORES)))
    return unshard(res.results)
